# revision 17
# baseline (speedup 1.0000x reference)
"""GAT (2-layer, 1-head) + linear readout on 8 NeuronCores, Bass/Tile.

Pure data-parallel over 32768 graphs (4096/core). The edge softmax runs densely
over the 14x14 (src,dst) matrix using per-graph edge counts (host-built from
edge_list), which reproduces the reference segment softmax exactly (duplicate
edges share one logit; counts weight the denominator and the aggregation).

v2: bf16 matmul operands (f32 accumulate, f32 scores/softmax), scores folded
into the layer matmul as two augmented output columns, pair-wide block-diag
scatter, DMA issue spread across SP/ACT/DVE sequencers.

Per pair of 128-graph tiles (32 block-diag groups of 8 graphs):
  [h_nat | sc] = [X_ct-chunk]^T @ [W | w_as | w_ad]  (PE, [128,130] per group)
  dense softmax in [graph-partition, 196] d-major layout (DVE/ACT)
  A = count * alpha -> DRAM -> block-diag scatter (8 slot DMAs per pair)
  T_ct = h_nat^T @ BD  (PE) -> +bias (relu on layer 1) -> bf16
  readout: 14 PSUM-accumulated MMs over (d,c), + bias, @Wp, sigmoid.
"""
import sys
sys.path.insert(0, '/opt/trn_rl_repo')
from contextlib import ExitStack

import numpy as np

import concourse.bacc as bacc
import concourse.bass as bass
import concourse.mybir as mybir
from concourse.tile import TileContext
from concourse import bass_utils

F32 = mybir.dt.float32
BF16 = mybir.dt.bfloat16
AL = mybir.AluOpType
AF = mybir.ActivationFunctionType

B, N, F, C, E = 32768, 14, 128, 128, 64
NCORES = 8
BC = B // NCORES            # graphs per core = 4096
BT = 128                    # graphs per tile
GP = 8                      # graphs per block-diag group
NG = BT // GP               # 16 groups per tile
NR = GP * N                 # 112 rows per group
BTN = BT * N                # 1792 node-cols per tile
RC = 512                    # readout chunk (graphs) = 4 tiles
NEG = 0.2
HC = 130                    # hnat-sb column pitch (128 h + 2 scores)
QS = (3, 3, 3, 3, 3, 1)     # group-chunking of the 16 groups per PSUM bank

_CACHE = {}


def build_kernel(bc=BC):
    npair = bc // (2 * BT)
    nc = bacc.Bacc("TRN2", target_bir_lowering=False)
    xct_d = nc.dram_tensor("xct", [128, bc * N], BF16, kind="ExternalInput")
    cnt_d = nc.dram_tensor("cnt", [bc, 196], BF16, kind="ExternalInput")
    waug1_d = nc.dram_tensor("waug1", [128, 130], BF16, kind="ExternalInput")
    waug2_d = nc.dram_tensor("waug2", [128, 130], BF16, kind="ExternalInput")
    b1_d = nc.dram_tensor("b1", [128, 1], F32, kind="ExternalInput")
    b2_d = nc.dram_tensor("b2", [128, 1], F32, kind="ExternalInput")
    wl_d = nc.dram_tensor("wl", [128, 14 * 64], BF16, kind="ExternalInput")
    bl_d = nc.dram_tensor("bl", [64, 1], F32, kind="ExternalInput")
    wp_d = nc.dram_tensor("wp", [64, 1], BF16, kind="ExternalInput")
    bp_d = nc.dram_tensor("bp", [1, 1], F32, kind="ExternalInput")
    pred_d = nc.dram_tensor("pred", [1, bc], F32, kind="ExternalOutput")
    attn_d = nc.dram_tensor("attn", [bc, 196], F32, kind="ExternalOutput")

    dma_engs = None
    _dc = [0]

    def dma(dst, src):
        eng = dma_engs[_dc[0] % len(dma_engs)]
        _dc[0] += 1
        eng.dma_start(dst, src)

    def dms(ap):  # [p, 2*196] -> [p, bt, d, s]
        return ap.rearrange("p (t d s) -> p t d s", t=2, d=14)

    with TileContext(nc) as tc, ExitStack() as ctx:
        dma_engs = [nc.sync, nc.scalar]
        cpool = ctx.enter_context(tc.tile_pool(name="consts", bufs=1))
        iop = ctx.enter_context(tc.tile_pool(name="io", bufs=4))
        smp = ctx.enter_context(tc.tile_pool(name="sm", bufs=4))
        actp = ctx.enter_context(tc.tile_pool(name="act", bufs=3))
        hnp = ctx.enter_context(tc.tile_pool(name="hn", bufs=4))
        h2p = ctx.enter_context(tc.tile_pool(name="h2", bufs=2))
        psh = ctx.enter_context(tc.tile_pool(name="ps_h", bufs=3, space="PSUM"))
        pst = ctx.enter_context(tc.tile_pool(name="ps_t", bufs=2, space="PSUM"))
        psro = ctx.enter_context(tc.tile_pool(name="ps_ro", bufs=2, space="PSUM"))
        pspr = ctx.enter_context(tc.tile_pool(name="ps_pr", bufs=1, space="PSUM"))
        dstp = ctx.enter_context(tc.tile_pool(name="dstage", bufs=6, space="DRAM"))

        waug1 = cpool.tile([128, 130], BF16)
        waug2 = cpool.tile([128, 130], BF16)
        b1 = cpool.tile([128, 1], F32)
        b2 = cpool.tile([128, 1], F32)
        wl = cpool.tile([128, 14 * 64], BF16)
        bl = cpool.tile([64, 1], F32)
        wp = cpool.tile([64, 1], BF16)
        bp = cpool.tile([1, 1], F32)
        zero = cpool.tile([128, 448], F32)
        for tile, dram in ((waug1, waug1_d), (waug2, waug2_d), (b1, b1_d),
                           (b2, b2_d), (wl, wl_d), (bl, bl_d), (wp, wp_d),
                           (bp, bp_d)):
            nc.sync.dma_start(tile[:, :], dram[:, :])
        nc.vector.memset(zero[:, :], 0.0)

        # Persistent pre-zeroed block-diag tiles (bf16, pair-wide: 32 groups);
        # the slot scatter overwrites exactly the diagonal blocks, so the
        # off-diagonal zeros persist across reuse.
        BDW = 2 * NG * NR  # 3584
        bd_tiles = []
        for i in range(4):  # [layer][pair parity]
            t = cpool.tile([NR, BDW], BF16, tag=f"bd{i}", name=f"bd{i}")
            nc.vector.memset(t[:, :], 0.0)
            bd_tiles.append(t)

        st = [dict() for _ in range(npair)]

        def s_load(pr):
            xct = [iop.tile([128, BTN + 16], BF16, tag=f"xct{t}", name=f"xct{t}")
                   for t in range(2)]
            for t in range(2):
                dma(xct[t][:, 0:BTN],
                    xct_d[:, (2 * pr + t) * BTN:(2 * pr + t + 1) * BTN])
                nc.vector.memset(xct[t][:, BTN:BTN + 16], 0.0)
            cnt = smp.tile([128, 392], BF16, tag="cnt")
            dma(cnt[:, :].rearrange("p (t j) -> p t j", t=2),
                bass.AP(tensor=cnt_d, offset=2 * pr * BT * 196,
                        ap=[[196, 128], [128 * 196, 2], [1, 196]]))
            if pr % (RC // (2 * BT)) == 0:
                st[pr]['h2ct'] = h2p.tile([128, RC * N], BF16, tag="h2ct")
            else:
                st[pr]['h2ct'] = st[pr - 1]['h2ct']
            st[pr]['xin'] = xct
            st[pr]['cnt'] = cnt

        def s_mm(pr, layer):
            waug = waug1 if layer == 0 else waug2
            xin = st[pr]['xin']
            hnat = [hnp.tile([NR, NG * 128], BF16, tag=f"hnat{t}",
                              name=f"hnat{t}") for t in range(2)]
            hsc = [smp.tile([NR, NG * 2], F32, tag=f"hsc{t}", name=f"hsc{t}")
                   for t in range(2)]
            for t in range(2):
                g0 = 0
                for qi, qn in enumerate(QS):
                    hps = psh.tile([128, 390], F32, tag="hps")
                    for g in range(qn):
                        gg = g0 + g
                        nc.tensor.matmul(
                            hps[:, 130 * g:130 * (g + 1)],
                            xin[t][:, NR * gg:NR * gg + 128],
                            waug[:, :], start=True, stop=True)
                    hv = hps[0:NR, :].rearrange("p (g c) -> p g c", g=3)[:, 0:qn, :]
                    if qi % 2:
                        nc.scalar.copy(
                            hnat[t][:, 128 * g0:128 * (g0 + qn)].rearrange(
                                "p (g c) -> p g c", g=qn),
                            hv[:, :, 0:128])
                    else:
                        nc.vector.tensor_copy(
                            hnat[t][:, 128 * g0:128 * (g0 + qn)].rearrange(
                                "p (g c) -> p g c", g=qn),
                            hv[:, :, 0:128])
                    nc.vector.tensor_copy(
                        hsc[t][:, 2 * g0:2 * (g0 + qn)].rearrange(
                            "p (g c) -> p g c", g=qn),
                        hv[:, :, 128:130])
                    g0 += qn
            scst = dstp.tile([2 * BT, 28], F32, tag="scst")
            for t in range(2):
                dma(bass.AP(tensor=scst.tensor,
                            offset=scst[:, :].offset + t * BT * 28,
                            ap=[[2, 112], [GP * 28, 16], [1, 2]]),
                    bass.AP(tensor=hsc[t].tensor,
                            offset=hsc[t][:, :].offset,
                            ap=[[NG * 2, 112], [2, 16], [1, 2]]))
            sc2 = smp.tile([128, 56], F32, tag="sc2")
            for t in range(2):
                dma(sc2[:, 28 * t:28 * t + 28].rearrange("p (n r) -> p n r", r=2),
                    bass.AP(tensor=scst.tensor,
                            offset=scst[:, :].offset + t * BT * 28,
                            ap=[[28, 128], [2, 14], [1, 2]]))
            st[pr]['hnat'] = hnat
            st[pr]['sc2'] = sc2

        def s_soft(pr, layer):
            sc2 = st[pr]['sc2']
            cnt = st[pr]['cnt']
            tl = smp.tile([128, 392], F32, tag="tl")
            as_ap = bass.AP(tensor=sc2.tensor, offset=sc2[:, :].offset,
                            ap=[[56, 128], [28, 2], [0, 14], [2, 14]])
            ad_ap = bass.AP(tensor=sc2.tensor, offset=sc2[:, :].offset + 1,
                            ap=[[56, 128], [28, 2], [2, 14], [0, 14]])
            nc.vector.tensor_tensor(dms(tl[:, :]), as_ap, ad_ap, AL.add)
            lr = smp.tile([128, 392], F32, tag="lr")
            nc.vector.scalar_tensor_tensor(lr[:, :], tl[:, :], NEG, tl[:, :],
                                           AL.mult, AL.max)
            ex = smp.tile([128, 392], F32, tag="ex")
            nc.scalar.activation(ex[:, :], lr[:, :], AF.Exp)
            ce = smp.tile([128, 392], F32, tag="ce")
            nc.vector.tensor_mul(ce[:, :], ex[:, :], cnt[:, :])
            zz = smp.tile([128, 28], F32, tag="zz")
            nc.vector.tensor_reduce(zz[:, :], dms(ce[:, :]),
                                    mybir.AxisListType.X, AL.add)
            zr = smp.tile([128, 28], F32, tag="zr")
            nc.vector.reciprocal(zr[:, :], zz[:, :])
            ez = smp.tile([128, 392], F32, tag="ez")
            zrb = bass.AP(tensor=zr.tensor, offset=zr[:, :].offset,
                          ap=[[28, 128], [14, 2], [1, 14], [0, 14]])
            nc.vector.tensor_tensor(dms(ez[:, :]), dms(ex[:, :]), zrb, AL.mult)
            aa = smp.tile([128, 392], BF16, tag="aa")
            a_sm = bass.AP(tensor=aa.tensor, offset=aa[:, :].offset,
                           ap=[[392, 128], [196, 2], [1, 14], [14, 14]])
            nc.vector.tensor_tensor(a_sm, dms(ez[:, :]), dms(cnt[:, :]), AL.mult)
            if layer == 1:
                mask = smp.tile([128, 392], F32, tag="mask")
                nc.vector.tensor_scalar_min(mask[:, :], cnt[:, :], 1.0)
                attn = smp.tile([128, 392], F32, tag="attn")
                nc.vector.tensor_mul(attn[:, :], ez[:, :], mask[:, :])
                dma(bass.AP(tensor=attn_d, offset=2 * pr * BT * 196,
                            ap=[[196, 128], [128 * 196, 2], [1, 196]]),
                    attn[:, :].rearrange("p (t j) -> p t j", t=2))
            ast = dstp.tile([2 * BT, 196], BF16, tag="ast")
            dma(bass.AP(tensor=ast.tensor, offset=ast[:, :].offset,
                        ap=[[196, 128], [128 * 196, 2], [1, 196]]),
                aa[:, :].rearrange("p (t j) -> p t j", t=2))
            bd = bd_tiles[2 * layer + pr % 2]
            for j in range(8):
                srcj = bass.AP(tensor=ast.tensor,
                               offset=ast[:, :].offset + j * 196,
                               ap=[[14, 14], [GP * 196, 2 * NG], [1, 14]])
                dstj = bass.AP(tensor=bd.tensor,
                               offset=bd[:, :].offset + (14 * j) * BDW + 14 * j,
                               ap=[[BDW, 14], [NR, 2 * NG], [1, 14]])
                if j % 2:
                    dma(dstj, srcj)
                else:
                    nc.gpsimd.dma_start(dstj, srcj)
            st[pr]['bd'] = bd

        def s_agg(pr, layer):
            bvec = b1 if layer == 0 else b2
            hnat = st[pr]['hnat']
            bd = st[pr]['bd']
            h2ct = None
            if layer == 1:
                if pr % (RC // (2 * BT)) == 0:
                    st[pr]['h2ct'] = h2p.tile([128, RC * N], BF16, tag="h2ct",
                                              name="h2ct")
                else:
                    st[pr]['h2ct'] = st[pr - 1]['h2ct']
                h2ct = st[pr]['h2ct']
            xout = [actp.tile([128, BTN + 16], BF16, tag=f"xout{t}",
                              name=f"xout{t}")
                    for t in range(2)] if layer == 0 else None
            for t in range(2):
                for q in range(4):
                    tps = pst.tile([128, 448], F32, tag="tps")
                    for g in range(4):
                        gg = 4 * q + g
                        nc.tensor.matmul(
                            tps[:, 112 * g:112 * (g + 1)],
                            hnat[t][:, 128 * gg:128 * gg + 128],
                            bd[:, NR * (NG * t + gg):NR * (NG * t + gg + 1)],
                            start=True, stop=True)
                    if layer == 0:
                        if q % 2:
                            nc.scalar.activation(
                                xout[t][:, 448 * q:448 * (q + 1)], tps[:, :],
                                AF.Relu, bias=bvec[:, 0:1])
                        else:
                            nc.vector.scalar_tensor_tensor(
                                xout[t][:, 448 * q:448 * (q + 1)], tps[:, :],
                                bvec[:, 0:1], zero[:, :], AL.add, AL.max)
                    else:
                        off = ((2 * pr + t) % (RC // BT)) * BTN
                        if q % 2:
                            nc.scalar.activation(
                                h2ct[:, off + 448 * q:off + 448 * (q + 1)],
                                tps[:, :], AF.Identity, bias=bvec[:, 0:1])
                        else:
                            nc.vector.tensor_scalar_add(
                                h2ct[:, off + 448 * q:off + 448 * (q + 1)],
                                tps[:, :], bvec[:, 0:1])
            if layer == 0:
                for t in range(2):
                    nc.vector.memset(xout[t][:, BTN:BTN + 16], 0.0)
                st[pr]['xin'] = xout

        def s_read(pr):
            if (2 * pr + 2) % (RC // BT) != 0:
                return
            h2ct = st[pr]['h2ct']
            rops = psro.tile([64, RC], F32, tag="rops")
            for d in range(14):
                nc.tensor.matmul(
                    rops[:, :], wl[:, 64 * d:64 * (d + 1)],
                    bass.AP(tensor=h2ct.tensor,
                            offset=h2ct[:, :].offset + d,
                            ap=[[RC * N, 128], [N, RC]]),
                    start=(d == 0), stop=(d == 13))
            xsb = actp.tile([64, RC], BF16, tag="xsb")
            nc.vector.tensor_scalar_add(xsb[:, :], rops[:, :], bl[:, 0:1])
            prps = pspr.tile([1, RC], F32, tag="prps")
            nc.tensor.matmul(prps[:, :], wp[:, :], xsb[:, :], start=True, stop=True)
            ck = (2 * pr + 2) * BT // RC - 1
            pred_sb = actp.tile([1, RC], F32, tag="pred_sb", name="pred_sb")
            nc.scalar.activation(pred_sb[:, :], prps[:, :],
                                 AF.Sigmoid, bias=bp[:, 0:1])
            dma(pred_d[:, RC * ck:RC * (ck + 1)], pred_sb[:, :])

        stages = [
            lambda p: (s_load(p), s_mm(p, 0)),
            lambda p: s_soft(p, 0),
            lambda p: (s_agg(p, 0), s_mm(p, 1)),
            lambda p: s_soft(p, 1),
            lambda p: (s_agg(p, 1), s_read(p)),
        ]
        for r in range(npair + len(stages) - 1):
            for si in range(len(stages)):
                p = r - si
                if 0 <= p < npair:
                    stages[si](p)
    nc.finalize()
    return nc


def _bf16_arr(x):
    """Return a bf16 numpy array (as ml_dtypes.bfloat16 if available)."""
    try:
        import ml_dtypes
        return np.asarray(x, np.float32).astype(ml_dtypes.bfloat16)
    except ImportError:
        import jax.numpy as jnp
        return np.asarray(jnp.asarray(np.asarray(x, np.float32), jnp.bfloat16))


def _prep(inputs):
    feature = np.asarray(inputs['feature'], np.float32)
    edge = np.asarray(inputs['edge_list'])
    W1 = np.asarray(inputs['W1'], np.float32)
    W2 = np.asarray(inputs['W2'], np.float32)
    Wl = np.asarray(inputs['Wl'], np.float32)
    waug1 = np.concatenate([
        W1,
        (W1.astype(np.float64) @ np.asarray(inputs['a_s1'], np.float64))[:, None].astype(np.float32),
        (W1.astype(np.float64) @ np.asarray(inputs['a_d1'], np.float64))[:, None].astype(np.float32)], 1)
    waug2 = np.concatenate([
        W2,
        (W2.astype(np.float64) @ np.asarray(inputs['a_s2'], np.float64))[:, None].astype(np.float32),
        (W2.astype(np.float64) @ np.asarray(inputs['a_d2'], np.float64))[:, None].astype(np.float32)], 1)
    src = edge[..., 0].astype(np.int64)
    dst = edge[..., 1].astype(np.int64)
    flat = (np.arange(B)[:, None] * 196 + dst * 14 + src).ravel()
    cnt = np.bincount(flat, minlength=B * 196).astype(np.float32).reshape(B, 196)
    cnt.reshape(B, 14, 14)[:, np.arange(14), np.arange(14)] += 1.0
    wl = np.ascontiguousarray(
        Wl.reshape(14, 128, 64).transpose(1, 0, 2).reshape(128, 14 * 64))
    common = dict(
        waug1=_bf16_arr(waug1), waug2=_bf16_arr(waug2),
        b1=np.asarray(inputs['b1'], np.float32).reshape(128, 1),
        b2=np.asarray(inputs['b2'], np.float32).reshape(128, 1),
        wl=_bf16_arr(wl),
        bl=np.asarray(inputs['bl'], np.float32).reshape(64, 1),
        wp=_bf16_arr(np.asarray(inputs['Wp'], np.float32).reshape(64, 1)),
        bp=np.asarray(inputs['bp'], np.float32).reshape(1, 1),
    )
    in_maps = []
    for k in range(NCORES):
        sl = slice(k * BC, (k + 1) * BC)
        xctk = _bf16_arr(np.ascontiguousarray(feature[sl].reshape(BC * N, F).T))
        in_maps.append(dict(xct=xctk, cnt=_bf16_arr(cnt[sl]), **common))
    return in_maps


def kernel(**inputs):
    if 'nc' not in _CACHE:
        _CACHE['nc'] = build_kernel()
    nc = _CACHE['nc']
    in_maps = _prep(inputs)
    res = bass_utils.run_bass_kernel_spmd(nc, in_maps, core_ids=list(range(NCORES)))
    preds, attns = [], []
    for k in range(NCORES):
        preds.append(res.results[k]['pred'].reshape(BC, 1))
        attns.append(res.results[k]['attn'].reshape(BC, 14, 14).transpose(0, 2, 1))
    return np.concatenate(preds, 0), np.concatenate(attns, 0)
